# revision 3
# baseline (speedup 1.0000x reference)
"""GQA sigmoid-attention (causal zero-fill) Trainium2 Bass kernel — v6.

v5 + fp8-e4m3 DoubleRow Q-projection: q and Wq ship as fp8 (Wq host-scaled
x32, absorbed into the sigmoid scale), each dt-PAIR contracts in one
DoubleRow matmul (256-deep, 0.5 cyc/row): Qproj drops 256->128 matmuls and
~22us; q DMA bytes halve. Measured end-to-end rel err ~1.3e-2 vs the 2e-2
gate (deterministic seeded inputs).

v3 + full software pipelining so the PE never idles waiting on ACT/DMA:
  - KV projection of tile j+1 is dripped into B(j) as PE filler work
    (its quad DMAs issue at B(j) start and prefetch under compute), so
    phase A(j>0) is only the Q projection.
  - C(j-1) matmuls drip 1-2 units after *every* AV matmul instead of as
    per-head blocks: ACT always has queued sigmoids, PE always has filler.
  - C accumulates in ps8-tagged PSUM pairs only; ps_kv holds the pipelined
    KV accumulators + xv transposes.
  - j=0 phase A interleaves KV+Q per dt-pair (PE tracks DMA arrival, wq
    arrives in dt-pair chunks just ahead of use).

Sharding: core = (b, g), b in {0,1} batches x g in {0..3} kv-groups; host
sums the 4 Wo-row-shard partials per batch (fp16 partials).
"""

import math

import numpy as np
import ml_dtypes

import concourse.bacc as bacc
import concourse.mybir as mybir
import concourse.tile as tile
from concourse.bass_utils import run_bass_kernel_spmd

B = 2
S = 2048
D = 2048
NH = 16
NKV = 4
C = 128          # head dim
HPG = NH // NKV  # 4 query heads per kv group (= per core)
F = HPG * C      # 512 query-proj dims per core
SCALE = 1.0 / math.sqrt(C)
P = 128
DT = D // P      # 16 contraction chunks
J4 = S // 512    # 4 query tiles of 512
ST = S // P      # 16 s-chunks

f32 = mybir.dt.float32
bf16 = mybir.dt.bfloat16
f16 = mybir.dt.float16
f8 = mybir.dt.float8e4
WQ_SCALE = 32.0

_CACHE: dict = {}
_PHASES = "ABC"


def _build_module(n_iters: int = 0, internal_io: bool = False):
    nc = bacc.Bacc("TRN2", target_bir_lowering=False, debug=False, num_devices=8)

    if internal_io:
        dummy_in = nc.dram_tensor("dummy_in", [1, 1], f32, kind="ExternalInput")
        dummy_out = nc.dram_tensor("dummy_out", [1, 1], f32, kind="ExternalOutput")
        kw = {}
    else:
        kw = {"kind": "ExternalInput"}
    qT = nc.dram_tensor("qT", [D, S], f8, **kw)
    kT = nc.dram_tensor("kT", [D, S], bf16, **kw)
    vT = nc.dram_tensor("vT", [D, S], bf16, **kw)
    wqP = nc.dram_tensor("wqP", [P, DT, F], f8, **kw)
    wkP = nc.dram_tensor("wkP", [P, DT, C], bf16, **kw)
    wvP = nc.dram_tensor("wvP", [P, DT, C], bf16, **kw)
    woP = nc.dram_tensor("woP", [P, HPG, D], bf16, **kw)
    maskP = nc.dram_tensor("maskP", [P, J4, 512], bf16, **kw)
    identP = nc.dram_tensor("identP", [P, P], bf16, **kw)
    if internal_io:
        out = nc.dram_tensor("out", [S, D], f16)
    else:
        out = nc.dram_tensor("out", [S, D], f16, kind="ExternalOutput")

    qT_r = qT.rearrange("(dt p) s -> p dt s", p=P)
    kT_r = kT.rearrange("(dt p) s -> p dt s", p=P)
    vT_r = vT.rearrange("(dt p) s -> p dt s", p=P)

    with tile.TileContext(nc) as tc:
        with (
            tc.tile_pool(name="consts", bufs=1) as consts,
            tc.tile_pool(name="weights", bufs=1) as wpool,
            tc.tile_pool(name="xkv", bufs=1) as xkv_pool,
            tc.tile_pool(name="xq", bufs=2) as xq_pool,
            tc.tile_pool(name="attn_sb", bufs=2) as apool,
            tc.tile_pool(name="qstream", bufs=3) as qstream,
            tc.tile_pool(name="kvstream", bufs=2) as kvstream,
            tc.tile_pool(name="q0stream", bufs=8) as q0stream,
            tc.tile_pool(name="vtr", bufs=2) as vtr,
            tc.tile_pool(name="probs", bufs=10) as probs,
            tc.tile_pool(name="oevac", bufs=6) as oevac,
            tc.tile_pool(name="ps_at", bufs=2, space="PSUM") as ps_at_pool,
            tc.tile_pool(name="ps_c", bufs=2, space="PSUM") as ps_c_pool,
            tc.tile_pool(name="ps_kv", bufs=2, space="PSUM") as ps_kv_pool,
            tc.tile_pool(name="ps_sc", bufs=2, space="PSUM") as ps_sc_pool,
        ):
          def body(_iv=None):
            ident = consts.tile([P, P], bf16, name="ident")
            masks = consts.tile([P, J4, 512], bf16, name="masks")

            wk_sb = wpool.tile([P, DT, C], bf16, tag="wk", name="wk_sb")
            wv_sb = wpool.tile([P, DT, C], bf16, tag="wv", name="wv_sb")
            wq_sb = wpool.tile([P, DT, F], f8, tag="wq", name="wq_sb")
            wo_sb = wpool.tile([P, HPG, D], bf16, tag="wo", name="wo_sb")

            # first-use order: wk/wv halves so the dt=0 matmuls start ~1us in
            nc.sync.dma_start(wk_sb[:, 0:DT // 2, :], wkP[:, 0:DT // 2, :])
            nc.sync.dma_start(wv_sb[:, 0:DT // 2, :], wvP[:, 0:DT // 2, :])

            xkT = xkv_pool.tile([P, S], bf16, tag="xkT", name="xkT")
            xv = xkv_pool.tile([P, ST, C], bf16, tag="xv", name="xv")

            def kv_evac(j, ps_k, ps_v):
                """ps_k/ps_v -> xkT column block + xv (PE transposes)."""
                nc.vector.tensor_copy(xkT[:, j * 512:(j + 1) * 512], ps_k[:])
                xvT_sb = vtr.tile([P, 512], bf16, tag="xvT", name="xvT_sb")
                nc.vector.tensor_copy(xvT_sb[:], ps_v[:])
                for sc in range(4):
                    pst = ps_kv_pool.tile([P, P], bf16, tag="kv", name="pst")
                    nc.tensor.transpose(pst[:], xvT_sb[:, sc * P:(sc + 1) * P],
                                        ident[:])
                    nc.vector.tensor_copy(xv[:, j * 4 + sc, :], pst[:])

            def kv_gen(j):
                """Generator: KV projection of tile j (DMAs upfront, then
                matmul drip units, then evac+transposes)."""
                sl_ = slice(j * 512, (j + 1) * 512)
                kcq = [kvstream.tile([P, 8, 512], bf16, tag="kc", name="kc")
                       for _ in range(2)]
                vcq = [kvstream.tile([P, 8, 512], bf16, tag="vc", name="vc")
                       for _ in range(2)]
                for hf in range(2):
                    nc.gpsimd.dma_start(kcq[hf][:], kT_r[:, 8 * hf:8 * hf + 8, sl_])
                    nc.gpsimd.dma_start(vcq[hf][:], vT_r[:, 8 * hf:8 * hf + 8, sl_])
                ps_k = ps_kv_pool.tile([P, 512], f32, tag="kv", name="ps_k")
                ps_v = ps_kv_pool.tile([P, 512], f32, tag="kv", name="ps_v")
                yield
                for hf in range(2):
                    for i in range(8):
                        dt = 8 * hf + i
                        st, sp = dt == 0, dt == DT - 1
                        nc.tensor.matmul(ps_k[:], wk_sb[:, dt, :],
                                         kcq[hf][:, i, :], start=st, stop=sp)
                        nc.tensor.matmul(ps_v[:], wv_sb[:, dt, :],
                                         vcq[hf][:, i, :], start=st, stop=sp)
                        yield
                kv_evac(j, ps_k, ps_v)
                yield

            def c_gen(at_prev, j):
                """Generator: C(j) output projection in drip units of 2
                matmuls (one ps_o pair, one head)."""
                for s16 in range(4):
                    row0 = (j * 4 + s16) * P
                    for np_ in range(2):
                        ps_o = [ps_c_pool.tile([P, 512], f32, tag="c",
                                                name=f"ps_o{i}")
                                for i in range(2)]
                        for h in range(HPG):
                            for i in range(2):
                                n4 = np_ * 2 + i
                                nc.tensor.matmul(
                                    ps_o[i][:],
                                    at_prev[:, h, s16 * P:(s16 + 1) * P],
                                    wo_sb[:, h, n4 * 512:(n4 + 1) * 512],
                                    start=(h == 0), stop=(h == HPG - 1))
                            yield
                        for i in range(2):
                            n4 = np_ * 2 + i
                            ot = oevac.tile([P, 512], f16, tag="ot", name="ot")
                            nc.vector.tensor_copy(ot[:], ps_o[i][:])
                            nc.sync.dma_start(
                                out[row0:row0 + P, n4 * 512:(n4 + 1) * 512],
                                ot[:])
                        yield

            def drain(gens, frac, state):
                """Advance each gen so that state[gen] reaches frac of its
                total unit budget."""
                for g, total in gens:
                    want = int(math.ceil(frac * total))
                    while state[id(g)] < want:
                        try:
                            next(g)
                        except StopIteration:
                            state[id(g)] = 10 ** 9
                            break
                        state[id(g)] += 1

            # ---------------- A(0): interleaved KV + Q per dt-pair
            ps_k0 = ps_kv_pool.tile([P, 512], f32, tag="kv", name="ps_k")
            ps_v0 = ps_kv_pool.tile([P, 512], f32, tag="kv", name="ps_v")
            ps_q = [ps_at_pool.tile([P, 512], f32, tag="at", name="psq0"),
                    ps_at_pool.tile([P, 512], f32, tag="at", name="psq1"),
                    ps_c_pool.tile([P, 512], f32, tag="c", name="psq2"),
                    ps_c_pool.tile([P, 512], f32, tag="c", name="psq3")]
            for dp in range(DT // 2):
                kc = kvstream.tile([P, 2, 512], bf16, tag="kc", name="kc")
                vc = kvstream.tile([P, 2, 512], bf16, tag="vc", name="vc")
                nc.gpsimd.dma_start(kc[:], kT_r[:, 2 * dp:2 * dp + 2, 0:512])
                nc.gpsimd.dma_start(vc[:], vT_r[:, 2 * dp:2 * dp + 2, 0:512])
                nc.sync.dma_start(wq_sb[:, 2 * dp:2 * dp + 2, :],
                                  wqP[:, 2 * dp:2 * dp + 2, :])
                qc = q0stream.tile([P, 2, 512], f8, tag="qc", name="qc")
                nc.sync.dma_start(qc[:], qT_r[:, 2 * dp:2 * dp + 2, 0:512])
                if dp == 1:
                    nc.sync.dma_start(wk_sb[:, DT // 2:, :], wkP[:, DT // 2:, :])
                    nc.sync.dma_start(wv_sb[:, DT // 2:, :], wvP[:, DT // 2:, :])
                for i in range(2):
                    dt = 2 * dp + i
                    st, sp = dt == 0, dt == DT - 1
                    nc.tensor.matmul(ps_k0[:], wk_sb[:, dt, :], kc[:, i, :],
                                     start=st, stop=sp)
                    nc.tensor.matmul(ps_v0[:], wv_sb[:, dt, :], vc[:, i, :],
                                     start=st, stop=sp)
                for h in range(HPG):
                    nc.tensor.matmul(
                        ps_q[h][:], wq_sb[:, 2 * dp:2 * dp + 2, h * P:(h + 1) * P],
                        qc[:], start=(dp == 0), stop=(dp == DT // 2 - 1),
                        perf_mode=mybir.MatmulPerfMode.DoubleRow)
            nc.sync.dma_start(ident[:], identP[:])
            nc.sync.dma_start(masks[:], maskP[:])
            nc.sync.dma_start(wo_sb[:], woP[:])
            kv_evac(0, ps_k0, ps_v0)
            xqT_j = xq_pool.tile([P, HPG, 512], bf16, tag="xqT", name="xqT_j")
            for h in range(HPG):
                nc.vector.tensor_copy(xqT_j[:, h, :], ps_q[h][:])

            def score_prob(j, xq_t, kc_i, h):
                r = kc_i - 4 * j
                c0 = 128 * r if r > 0 else 0
                ps_s = ps_sc_pool.tile([P, 512], f32, tag="sc", name="ps_s")
                nc.tensor.matmul(
                    ps_s[:, c0:], xkT[:, kc_i * P:(kc_i + 1) * P],
                    xq_t[:, h, c0:], start=True, stop=True)
                pr = probs.tile([P, 512], bf16, tag="pr", name="pr")
                nc.scalar.activation(
                    pr[:, c0:], ps_s[:, c0:],
                    mybir.ActivationFunctionType.Sigmoid,
                    scale=float(SCALE / WQ_SCALE))
                if r >= 0:
                    nc.vector.tensor_mul(
                        out=pr[:, c0:], in0=pr[:, c0:], in1=masks[:, r, c0:])
                return pr, c0

            at_prev = None
            for j in range(J4):
                nk = 4 * (j + 1)
                at_block = apool.tile([P, HPG, 512], bf16, tag="attnT",
                                      name="at_block")
                # filler generators dripped through B(j)
                gens = []
                qcq_next = []
                if j + 1 < J4:
                    def q_dma_gen(jn=j + 1):
                        sl2 = slice(jn * 512, (jn + 1) * 512)
                        for hf in range(2):
                            t = qstream.tile([P, 8, 512], f8, tag="qc",
                                             name="qc")
                            nc.sync.dma_start(t[:],
                                              qT_r[:, 8 * hf:8 * hf + 8, sl2])
                            qcq_next.append(t)
                        yield
                    gens.append((q_dma_gen(), 1))
                if at_prev is not None and "C" in _PHASES:
                    gens.append((c_gen(at_prev, j - 1), 40))
                if j + 1 < J4:
                    gens.append((kv_gen(j + 1), 18))
                state = {id(g): 0 for g, _ in gens}
                npts = nk * HPG
                pt = 0
                drain(gens, 2.5 / npts, state)

                if "B" not in _PHASES:
                    drain(gens, 1.0, state)
                elif j == 0:
                    # B(0): kc-outer, all heads in flight
                    ps_at = [ps_at_pool.tile([P, 512], f32, tag="at",
                                              name="ps_at0"),
                             ps_at_pool.tile([P, 512], f32, tag="at",
                                             name="ps_at1"),
                             ps_c_pool.tile([P, 512], f32, tag="c",
                                            name="ps_at2"),
                             ps_c_pool.tile([P, 512], f32, tag="c",
                                            name="ps_at3")]
                    for kc_i in range(nk):
                        prs = [score_prob(j, xqT_j, kc_i, h) for h in range(HPG)]
                        for h in range(HPG):
                            pr, c0 = prs[h]
                            nc.tensor.matmul(ps_at[h][:, c0:], xv[:, kc_i, :],
                                             pr[:, c0:], start=(kc_i == 0),
                                             stop=(kc_i == nk - 1))
                            pt += 1
                            drain(gens, pt / npts, state)
                    for h in range(HPG):
                        nc.vector.tensor_copy(at_block[:, h, :], ps_at[h][:])
                else:
                    for h in range(HPG):
                        ps_at = ps_at_pool.tile([P, 512], f32, tag="at",
                                                name="ps_at")
                        for kc_i in range(nk):
                            pr, c0 = score_prob(j, xqT_j, kc_i, h)
                            nc.tensor.matmul(ps_at[:, c0:], xv[:, kc_i, :],
                                             pr[:, c0:], start=(kc_i == 0),
                                             stop=(kc_i == nk - 1))
                            pt += 1
                            drain(gens, pt / npts, state)
                        nc.vector.tensor_copy(at_block[:, h, :], ps_at[:])
                drain(gens, 1.0, state)
                at_prev = at_block

                # ---- A(j+1): Q projection only (qcq quads prefetched)
                if j + 1 < J4:
                    qcq = qcq_next
                    xqT_j = xq_pool.tile([P, HPG, 512], bf16, tag="xqT",
                                         name="xqT_j")
                    ps_q = [ps_at_pool.tile([P, 512], f32, tag="at", name="psq0"),
                            ps_at_pool.tile([P, 512], f32, tag="at", name="psq1"),
                            ps_c_pool.tile([P, 512], f32, tag="c", name="psq2"),
                            ps_c_pool.tile([P, 512], f32, tag="c", name="psq3")]
                    for hf in range(2):
                        for i2 in range(4):
                            dtp = 8 * hf + 2 * i2
                            st = hf == 0 and i2 == 0
                            sp = hf == 1 and i2 == 3
                            for h in range(HPG):
                                nc.tensor.matmul(
                                    ps_q[h][:],
                                    wq_sb[:, dtp:dtp + 2, h * P:(h + 1) * P],
                                    qcq[hf][:, 2 * i2:2 * i2 + 2, :],
                                    start=st, stop=sp,
                                    perf_mode=mybir.MatmulPerfMode.DoubleRow)
                    for h in range(HPG):
                        nc.vector.tensor_copy(xqT_j[:, h, :], ps_q[h][:])

            if "C" in _PHASES:
                for g, _ in [(c_gen(at_prev, J4 - 1), 40)]:
                    for _ in g:
                        pass

          if internal_io:
              dt_ = consts.tile([1, 1], f32, name="dt_")
              nc.sync.dma_start(dt_[:], dummy_in[:])
              nc.sync.dma_start(dummy_out[:], dt_[:])
          if n_iters:
              with tc.For_i(0, n_iters, 1):
                  body()
          else:
              body()
    nc.compile()
    return nc


def _get_module():
    if "nc" not in _CACHE:
        _CACHE["nc"] = _build_module()
    return _CACHE["nc"]


def _host_masks() -> np.ndarray:
    """mask_r[i, r, jq] = 1 iff jq - i - 128 r >= 0 (keep k <= q)."""
    i = np.arange(P)[:, None, None]
    r = np.arange(J4)[None, :, None]
    jq = np.arange(512)[None, None, :]
    return (jq - i - P * r >= 0).astype(ml_dtypes.bfloat16)


def _pack_w(wT: np.ndarray, free: int) -> np.ndarray:
    return np.ascontiguousarray(
        wT.reshape(DT, P, free).transpose(1, 0, 2)).astype(ml_dtypes.bfloat16)


def _pack_w8(wT: np.ndarray, free: int) -> np.ndarray:
    return np.ascontiguousarray(
        wT.reshape(DT, P, free).transpose(1, 0, 2)).astype(
            ml_dtypes.float8_e4m3)


def make_in_maps(query, key, value, Wq, Wk, Wv, Wo):
    bf = ml_dtypes.bfloat16
    query = np.asarray(query, dtype=np.float32)
    key = np.asarray(key, dtype=np.float32)
    value = np.asarray(value, dtype=np.float32)
    Wq = np.asarray(Wq, dtype=np.float32)
    Wk = np.asarray(Wk, dtype=np.float32)
    Wv = np.asarray(Wv, dtype=np.float32)
    Wo = np.asarray(Wo, dtype=np.float32)

    f8h = ml_dtypes.float8_e4m3
    qT = [np.ascontiguousarray(query[b].T).astype(f8h) for b in range(B)]
    kTb = [np.ascontiguousarray(key[b].T).astype(bf) for b in range(B)]
    vTb = [np.ascontiguousarray(value[b].T).astype(bf) for b in range(B)]
    WqT = Wq.T
    WkT = Wk.T
    WvT = Wv.T
    WoT = Wo.T
    masks = _host_masks()
    ident = np.eye(P, dtype=ml_dtypes.bfloat16)

    in_maps = []
    for core in range(8):
        b, g = divmod(core, 4)
        woT_g = WoT[g * F:(g + 1) * F, :]
        in_maps.append({
            "qT": qT[b],
            "kT": kTb[b],
            "vT": vTb[b],
            "wqP": _pack_w8(WqT[:, g * F:(g + 1) * F] * WQ_SCALE, F),
            "wkP": _pack_w(WkT[:, g * C:(g + 1) * C], C),
            "wvP": _pack_w(WvT[:, g * C:(g + 1) * C], C),
            "woP": np.ascontiguousarray(
                woT_g.reshape(HPG, P, D).transpose(1, 0, 2)).astype(bf),
            "maskP": masks,
            "identP": ident,
        })
    return in_maps


def kernel(**inputs) -> np.ndarray:
    nc = _get_module()
    in_maps = make_in_maps(**inputs)
    res = run_bass_kernel_spmd(nc, in_maps, core_ids=list(range(8)))
    parts = [np.asarray(res.results[c]["out"], dtype=np.float32)
             for c in range(8)]
    full = np.empty((B, S, D), dtype=np.float32)
    for b in range(B):
        full[b] = parts[b * 4] + parts[b * 4 + 1] + parts[b * 4 + 2] + parts[b * 4 + 3]
    return full
